# revision 1
# baseline (speedup 1.0000x reference)
"""CSwin vertical-stripe window attention (sparse_attention) on 8 TRN2 cores.

Sharding: data-parallel over batch B=8 (one image per NeuronCore). No
collectives. Per-core kernel computes windowed attention + LePE depthwise
conv + output projection for one [4096, 256] image.

Layout choices (see window token order t' = s*64 + h, column-major within
the vertical stripe so the shifted-window mask becomes two contiguous
halves):
 - qT/kT: [c, t'] via PE transposes; QK^T in fp32r, 4-head row-packed
   (tile_position) into one 4-bank PSUM tile; one batched Exp (N=2048).
 - mask (window 7 only): memset of masked quadrant halves of expT.
 - PV: bf16, 4-head col-packed, attnT consumed directly as moving operand.
 - softmax denominators: ones-matmul col-packed M=1; reciprocal on DVE;
   broadcast to 128 partitions via a K=4 block-indicator matmul.
 - LePE: depthwise 3x3 as 9 diagonal bf16 matmuls over a zero-guarded
   padded vT layout (pad col per 64-row stripe column kills all branch
   logic at window edges).
 - proj: bf16 matmuls, bias added via a K=1 ones-row matmul.
"""
import os
import numpy as np
import ml_dtypes

import concourse.bass as bass
import concourse.bacc as bacc
import concourse.mybir as mybir
import concourse.tile as tile

RESO, STRIPE, DIM, NH, HD = 64, 8, 256, 8, 32
B, L, WIN, NW = 8, RESO * RESO, RESO * STRIPE, RESO // STRIPE
P = 128
F32, BF16, F32R = mybir.dt.float32, mybir.dt.bfloat16, mybir.dt.float32r
SEG = RESO + 1          # 65: padded stripe-column stride (h plus one pad)
GUARD = SEG + 1         # 66: leading/trailing zero guard
VPD = STRIPE * SEG      # 520 data cols
VPT = GUARD + VPD + GUARD  # 652 total padded vT cols
HALF = VPD // 2         # 260 (one PSUM bank at fp32 is 512; 260 fits)

Exp = mybir.ActivationFunctionType.Exp


def _r(ap):
    return ap.bitcast(F32R)


def build_nc():
    nc = bacc.Bacc("TRN2", target_bir_lowering=False, debug=False)
    q = nc.declare_dram_parameter("q", [L, DIM], F32, isOutput=False)
    k = nc.declare_dram_parameter("k", [L, DIM], F32, isOutput=False)
    v = nc.declare_dram_parameter("v", [L, DIM], BF16, isOutput=False)
    pw = nc.declare_dram_parameter("pw", [DIM, DIM], BF16, isOutput=False)
    pb = nc.declare_dram_parameter("pb", [1, DIM], BF16, isOutput=False)
    ld = nc.declare_dram_parameter("ld", [18, P, P], BF16, isOutput=False)
    idf_d = nc.declare_dram_parameter("idf", [P, P], F32, isOutput=False)
    idb_d = nc.declare_dram_parameter("idb", [P, P], BF16, isOutput=False)
    out = nc.declare_dram_parameter("out", [L, DIM], F32, isOutput=True)

    # window views: l = h*64 + w*8 + s ; token order t' = s*64 + h
    qv = q[:].rearrange("(h w s2 s1) c -> w s1 h s2 c", h=RESO, w=NW, s2=4, s1=2)
    kv = k[:].rearrange("(h w s2 s1) c -> w s1 h s2 c", h=RESO, w=NW, s2=4, s1=2)
    vv = v[:].rearrange("(h w s2 s1) c -> w s1 h s2 c", h=RESO, w=NW, s2=4, s1=2)
    ov = out[:].rearrange("(h w s2 s1) c -> w s2 s1 h c", h=RESO, w=NW, s2=4, s1=2)

    with tile.TileContext(nc) as tc:
        with tc.tile_pool(name="const", bufs=1) as cp, \
             tc.tile_pool(name="sb", bufs=1) as sp, \
             tc.tile_pool(name="ps", bufs=1, space="PSUM") as pp:
            # ---- constants ----
            idf = cp.tile([P, P], F32, name="idf")
            nc.sync.dma_start(idf[:], idf_d[:])
            idb = cp.tile([P, P], BF16, name="idb")
            nc.sync.dma_start(idb[:], idb_d[:])
            ones32 = cp.tile([P, 32], BF16, name="ones32")
            nc.vector.memset(ones32[:], 1.0)
            ones_row = cp.tile([1, P], BF16, name="ones_row")
            nc.vector.memset(ones_row[:], 1.0)
            pw_sb = cp.tile([P, 2, DIM], BF16, name="pw_sb")
            for a in range(2):
                nc.sync.dma_start(pw_sb[:, a, :], pw[P * a:P * (a + 1), :])
            pb_sb = cp.tile([1, DIM], BF16, name="pb_sb")
            nc.sync.dma_start(pb_sb[:], pb[:])
            ld_sb = cp.tile([P, 18, P], BF16, name="ld_sb")
            for t in range(18):
                nc.sync.dma_start(ld_sb[:, t, :], ld[:][t])

            for w in range(NW):
                # ---- load window (nested AP: partition = s1*64+h) ----
                qn = sp.tile([P, 4, DIM], F32, name=f"qn{w}", tag="qn", bufs=2)
                kn = sp.tile([P, 4, DIM], F32, name=f"kn{w}", tag="kn", bufs=2)
                vn = sp.tile([P, 4, DIM], BF16, name=f"vn{w}", tag="vn", bufs=2)
                for t_, src in ((qn, qv), (kn, kv), (vn, vv)):
                    for s1 in range(2):
                        nc.sync.dma_start(
                            t_[RESO * s1:RESO * (s1 + 1), :, :], src[w, s1])

                # ---- transposes ----
                qT, kT, vTp = [], [], []
                for cc in range(2):
                    pt = pp.tile([P, 512], F32, name=f"tq{w}{cc}", tag="aux", bufs=1)
                    for t4 in range(4):
                        nc.tensor.transpose(pt[:, P * t4:P * (t4 + 1)],
                                            qn[:, t4, P * cc:P * (cc + 1)], idf[:])
                    qt = sp.tile([P, 512], F32R, name=f"qT{w}{cc}", tag="qT", bufs=4)
                    nc.vector.tensor_copy(qt[:], pt[:])
                    qT.append(qt)
                for cc in range(2):
                    pt = pp.tile([P, 512], F32, name=f"tk{w}{cc}", tag="aux", bufs=1)
                    for t4 in range(4):
                        nc.tensor.transpose(pt[:, P * t4:P * (t4 + 1)],
                                            kn[:, t4, P * cc:P * (cc + 1)], idf[:])
                    kt = sp.tile([P, 512], F32R, name=f"kT{w}{cc}", tag="kT", bufs=4)
                    nc.vector.tensor_copy(kt[:], pt[:])
                    kT.append(kt)
                for cc in range(2):
                    ptf = pp.tile([P, 512], F32, name=f"tv{w}{cc}", tag="aux", bufs=1)
                    pt = ptf[:, 0:256].bitcast(BF16)
                    for t4 in range(4):
                        nc.tensor.transpose(pt[:, P * t4:P * (t4 + 1)],
                                            vn[:, t4, P * cc:P * (cc + 1)], idb[:])
                    vt = sp.tile([P, VPT], BF16, name=f"vT{w}{cc}", tag="vTp", bufs=4)
                    nc.vector.memset(vt[:], 0.0)
                    nc.vector.tensor_copy(
                        vt[:, GUARD:GUARD + VPD].rearrange(
                            "p (s x) -> p s x", s=STRIPE)[:, :, :RESO],
                        pt.rearrange("p (s h) -> p s h", s=STRIPE))
                    vTp.append(vt)

                merged = []
                for g in range(2):
                    # ---- QK^T (fp32r, 4-head row-packed) + batched exp ----
                    eTs = []
                    for jc in range(4):
                        big = pp.tile([P, 2048], F32, name=f"bg{w}{g}{jc}",
                                      tag="big", bufs=1)
                        for hp in range(4):
                            nc.tensor.matmul(
                                big[:, 512 * hp:512 * (hp + 1)],
                                kT[g][32 * hp:32 * hp + 32, P * jc:P * (jc + 1)],
                                qT[g][32 * hp:32 * hp + 32, :],
                                start=True, stop=True, tile_position=(32 * hp, 0))
                        eT = sp.tile([P, 2048], BF16, name=f"eT{w}{g}{jc}",
                                     tag="eT", bufs=6)
                        nc.scalar.activation(eT[:], big[:], Exp, bias=0.0, scale=1.0)
                        if w == NW - 1:
                            for hp in range(4):
                                if jc < 2:
                                    nc.vector.memset(
                                        eT[:, 512 * hp + 256:512 * hp + 512], 0.0)
                                else:
                                    nc.vector.memset(
                                        eT[:, 512 * hp:512 * hp + 256], 0.0)
                        eTs.append(eT)

                    # ---- PV (bf16 col-packed) + denominators ----
                    pv = pp.tile([P, 512], F32, name=f"pv{w}{g}", tag="acc", bufs=2)
                    sm = pp.tile([P, 512], F32, name=f"sm{w}{g}", tag="acc", bufs=2)
                    for hp in range(4):
                        for jc in range(4):
                            nc.tensor.matmul(
                                pv[32 * hp:32 * hp + 32, :],
                                vn[:, jc, P * g + 32 * hp:P * g + 32 * hp + 32],
                                eTs[jc][:, 512 * hp:512 * (hp + 1)],
                                start=(jc == 0), stop=(jc == 3),
                                tile_position=(0, 32 * hp))
                        for jc in range(4):
                            nc.tensor.matmul(
                                sm[32 * hp:32 * hp + 32, :],
                                ones32[:],
                                eTs[jc][:, 512 * hp:512 * (hp + 1)],
                                start=(jc == 0), stop=(jc == 3),
                                tile_position=(0, 32 * hp))

                    rbs = sp.tile([P, 512], F32, name=f"rbs{w}{g}", tag="rbs", bufs=2)
                    nc.vector.reciprocal(rbs[:], sm[:])

                    # ---- LePE (9 diagonal bf16 matmuls per half) + merge ----
                    mg = sp.tile([P, 512], BF16, name=f"mg{w}{g}", tag="mg", bufs=4)
                    for half in range(2):
                        lp = pp.tile([P, HALF], F32, name=f"lp{w}{g}{half}",
                                     tag="lepe", bufs=1)
                        for tap in range(9):
                            dy, dx = tap // 3 - 1, tap % 3 - 1
                            so = GUARD + HALF * half + SEG * dx + dy
                            nc.tensor.matmul(
                                lp[:], ld_sb[:, 9 * g + tap, :],
                                vTp[g][:, so:so + HALF],
                                start=(tap == 0), stop=(tap == 8))
                        tmp = sp.tile([P, 256], F32, name=f"mt{w}{g}{half}",
                                      tag="mt", bufs=2)
                        nc.vector.tensor_tensor(
                            out=tmp[:], in0=pv[:, 256 * half:256 * (half + 1)],
                            in1=rbs[:, 256 * half:256 * (half + 1)],
                            op=mybir.AluOpType.mult)
                        nc.vector.tensor_tensor(
                            out=mg[:, 256 * half:256 * (half + 1)].rearrange(
                                "p (s x) -> p s x", s=4),
                            in0=tmp[:].rearrange("p (s x) -> p s x", s=4),
                            in1=lp[:].rearrange(
                                "p (s x) -> p s x", s=4)[:, :, :RESO],
                            op=mybir.AluOpType.add)
                    merged.append(mg)

                # ---- proj (bf16) + bias via K=1 matmul ----
                for t4 in range(4):
                    pj = pp.tile([P, DIM], F32, name=f"pj{w}{t4}", tag="aux", bufs=1)
                    nc.tensor.matmul(pj[:], merged[0][:, P * t4:P * (t4 + 1)],
                                     pw_sb[:, 0, :], start=True, stop=False)
                    nc.tensor.matmul(pj[:], merged[1][:, P * t4:P * (t4 + 1)],
                                     pw_sb[:, 1, :], start=False, stop=False)
                    nc.tensor.matmul(pj[:], ones_row[:], pb_sb[:],
                                     start=False, stop=True)
                    ob = sp.tile([P, DIM], F32, name=f"ob{w}{t4}", tag="ob", bufs=3)
                    nc.vector.tensor_copy(ob[:], pj[:])
                    for s1 in range(2):
                        nc.sync.dma_start(ov[w, t4, s1],
                                          ob[RESO * s1:RESO * (s1 + 1), :])
    return nc


_CACHE = {}


def _get_nc():
    if "nc" not in _CACHE:
        nc = build_nc()
        nc.finalize()
        _CACHE["nc"] = nc
    return _CACHE["nc"]


def _host_prep(qkv, scale, proj_w, proj_b, conv_w, conv_b):
    """Per-core input maps (host-side weight folding + batch shard)."""
    scale_v = float(np.asarray(scale).reshape(-1)[0])
    q_all = (np.asarray(qkv[0]) * scale_v).astype(np.float32)
    k_all = np.asarray(qkv[1]).astype(np.float32)
    v_all = np.asarray(qkv[2]).astype(ml_dtypes.bfloat16)
    pw_h = np.ascontiguousarray(np.asarray(proj_w).T).astype(ml_dtypes.bfloat16)
    # fold conv bias through the projection: out += (conv_b @ proj_w.T)
    pb_h = (np.asarray(proj_b) +
            np.asarray(conv_b) @ np.asarray(proj_w).T).astype(ml_dtypes.bfloat16)
    pb_h = pb_h.reshape(1, DIM)
    ldm = np.zeros((18, P, P), np.float32)
    cw = np.asarray(conv_w).reshape(DIM, 3, 3)
    for cc in range(2):
        for tap in range(9):
            dy, dx = tap // 3, tap % 3
            np.fill_diagonal(ldm[9 * cc + tap], cw[P * cc:P * (cc + 1), dy, dx])
    ldm = ldm.astype(ml_dtypes.bfloat16)
    idf_h = np.eye(P, dtype=np.float32)
    idb_h = np.eye(P, dtype=ml_dtypes.bfloat16)
    in_maps = []
    for b in range(B):
        in_maps.append({
            "q": np.ascontiguousarray(q_all[b]),
            "k": np.ascontiguousarray(k_all[b]),
            "v": np.ascontiguousarray(v_all[b]),
            "pw": pw_h, "pb": pb_h, "ld": ldm, "idf": idf_h, "idb": idb_h,
        })
    return in_maps


LAST_RESULTS = None


def kernel(qkv, scale, proj_w, proj_b, conv_w, conv_b):
    global LAST_RESULTS
    from concourse.bass_utils import run_bass_kernel_spmd
    nc = _get_nc()
    in_maps = _host_prep(qkv, scale, proj_w, proj_b, conv_w, conv_b)
    res = run_bass_kernel_spmd(nc, in_maps, core_ids=list(range(B)))
    LAST_RESULTS = res
    outs = [np.asarray(res.results[b]["out"], dtype=np.float32) for b in range(B)]
    return np.stack(outs, axis=0)



# revision 3
# speedup vs baseline: 1.9649x; 1.9649x over previous
"""CSwin vertical-stripe window attention (sparse_attention) on 8 TRN2 cores.

Sharding: data-parallel over batch B=8 (one image per NeuronCore). No
collectives. Per-core kernel computes windowed attention + LePE depthwise
conv + output projection for one [4096, 256] image.

v2 design (vs v1 baseline at ~496 us):
 - All transposes moved to HOST: q/k pre-transposed per-window to
   [w, g, c, j] (j = s2*128 + s1*64 + h token order), v prepared both as
   [w, p, jc, c] (PV stationary) and as the zero-padded LePE layout
   [w, g, c, 66 + s*65 + h]. Kills 192 PE transposes + PSUM aux traffic
   + DVE cast/copies.
 - QK^T in bf16 (was f32r), 4-head row-packed into one [128,2048] PSUM
   tile; one batched Exp (N=2048) per (g,jc).
 - PV + softmax-denominator (ones) matmuls in bf16 col-packed as before.
 - LePE on the DVE: 9 fused scalar_tensor_tensor taps (per-partition tap
   weight) over the padded layout; frees ~55us of PE time.
 - reciprocal_approx_fast for denominators (~5x faster than reciprocal).
 - Output copies PSUM->SBUF and w7 mask memsets on the idle GPSIMD.
 - Software pipelining: QK of slot (w,g) is interleaved with PV/sm of
   slot (w,g-1) so the PE stays busy while Exp produces eT.
"""
import numpy as np
import ml_dtypes

import concourse.bass as bass
import concourse.bacc as bacc
import concourse.mybir as mybir
import concourse.tile as tile

RESO, STRIPE, DIM, NH, HD = 64, 8, 256, 8, 32
B, L, WIN, NW = 8, RESO * RESO, RESO * STRIPE, RESO // STRIPE
P = 128
F32, BF16 = mybir.dt.float32, mybir.dt.bfloat16
SEG = RESO + 1          # 65: padded stripe-column stride (h plus one pad)
GUARD = SEG + 1         # 66: leading/trailing zero guard
VPD = STRIPE * SEG      # 520 data cols
VPT = GUARD + VPD + GUARD  # 652 total padded vT cols

Exp = mybir.ActivationFunctionType.Exp
MUL = mybir.AluOpType.mult
ADD = mybir.AluOpType.add


class Slot:
    def __init__(self, w, g, t):
        self.w, self.g, self.t = w, g, t
        self.eTs = [None] * 4
        self.pv = self.sm = self.mg = self.acc = None


def build_nc():
    nc = bacc.Bacc("TRN2", target_bir_lowering=False, debug=False)
    qT = nc.declare_dram_parameter("qT", [NW, 2, P, WIN], BF16, isOutput=False)
    kT = nc.declare_dram_parameter("kT", [NW, 2, P, WIN], BF16, isOutput=False)
    vn = nc.declare_dram_parameter("vn", [NW, P, 4, DIM], BF16, isOutput=False)
    vT = nc.declare_dram_parameter("vT", [NW, 2, P, VPT], BF16, isOutput=False)
    pw = nc.declare_dram_parameter("pw", [DIM, DIM], BF16, isOutput=False)
    pb = nc.declare_dram_parameter("pb", [1, DIM], BF16, isOutput=False)
    ld = nc.declare_dram_parameter("ld", [P, 18], F32, isOutput=False)
    out = nc.declare_dram_parameter("out", [L, DIM], F32, isOutput=True)

    # out view: l = h*64 + w*8 + s2*2 + s1 ; token j = s2*128 + s1*64 + h
    ov = out[:].rearrange("(h w s2 s1) c -> w s2 s1 h c", h=RESO, w=NW, s2=4, s1=2)

    with tile.TileContext(nc) as tc:
        with tc.tile_pool(name="const", bufs=1) as cp, \
             tc.tile_pool(name="sb", bufs=1) as sp, \
             tc.tile_pool(name="ps", bufs=1, space="PSUM") as pp:
            # ---- constants ----
            ones32 = cp.tile([P, 32], BF16, name="ones32")
            nc.vector.memset(ones32[:], 1.0)
            ones_row = cp.tile([1, P], BF16, name="ones_row")
            nc.vector.memset(ones_row[:], 1.0)
            pw_sb = cp.tile([P, 2, DIM], BF16, name="pw_sb")
            for a in range(2):
                nc.sync.dma_start(pw_sb[:, a, :], pw[P * a:P * (a + 1), :])
            pb_sb = cp.tile([1, DIM], BF16, name="pb_sb")
            nc.sync.dma_start(pb_sb[:], pb[:])
            ld_sb = cp.tile([P, 18], F32, name="ld_sb")
            nc.sync.dma_start(ld_sb[:], ld[:])

            def load_w(w):
                qt = sp.tile([P, 2, WIN], BF16, name=f"qt{w}", tag="qt", bufs=2)
                nc.sync.dma_start(qt[:], qT[:][w].rearrange("g c j -> c g j"))
                kt = sp.tile([P, 2, WIN], BF16, name=f"kt{w}", tag="kt", bufs=2)
                nc.sync.dma_start(kt[:], kT[:][w].rearrange("g c j -> c g j"))
                vt = sp.tile([P, 4, DIM], BF16, name=f"vt{w}", tag="vt", bufs=2)
                nc.sync.dma_start(vt[:], vn[:][w])
                vp = sp.tile([P, 2, VPT], BF16, name=f"vp{w}", tag="vp", bufs=2)
                nc.sync.dma_start(vp[:], vT[:][w].rearrange("g c t -> c g t"))
                return {"qt": qt, "kt": kt, "vt": vt, "vp": vp}

            def emit_qk(cur, jc):
                big = pp.tile([P, 2048], F32, name=f"bg{cur.w}{cur.g}{jc}",
                              tag="big", bufs=1)
                qt, kt = cur.t["qt"], cur.t["kt"]
                for hp in range(4):
                    nc.tensor.matmul(
                        big[:, 512 * hp:512 * (hp + 1)],
                        kt[32 * hp:32 * hp + 32, cur.g, P * jc:P * (jc + 1)],
                        qt[32 * hp:32 * hp + 32, cur.g, :],
                        start=True, stop=True, tile_position=(32 * hp, 0))
                eT = sp.tile([P, 2048], BF16, name=f"eT{cur.w}{cur.g}{jc}",
                             tag="eT", bufs=8)
                nc.scalar.activation(eT[:], big[:], Exp, bias=0.0, scale=1.0)
                if cur.w == NW - 1:
                    # shifted-window mask: zero the cross-half quadrants
                    for hp in range(4):
                        if jc < 2:
                            nc.gpsimd.memset(
                                eT[:, 512 * hp + 256:512 * hp + 512], 0.0)
                        else:
                            nc.gpsimd.memset(
                                eT[:, 512 * hp:512 * hp + 256], 0.0)
                cur.eTs[jc] = eT

            def emit_pvsm(cur, jc):
                if jc == 0:
                    cur.pv = pp.tile([P, WIN], F32, name=f"pv{cur.w}{cur.g}",
                                     tag="pv", bufs=1)
                    cur.sm = pp.tile([P, WIN], F32, name=f"sm{cur.w}{cur.g}",
                                     tag="sm", bufs=1)
                vt = cur.t["vt"]
                eT = cur.eTs[jc]
                for hp in range(4):
                    nc.tensor.matmul(
                        cur.pv[32 * hp:32 * hp + 32, :],
                        vt[:, jc, P * cur.g + 32 * hp:P * cur.g + 32 * hp + 32],
                        eT[:, 512 * hp:512 * (hp + 1)],
                        start=(jc == 0), stop=(jc == 3),
                        tile_position=(0, 32 * hp), skip_group_check=True)
                for hp in range(4):
                    nc.tensor.matmul(
                        cur.sm[32 * hp:32 * hp + 32, :],
                        ones32[:],
                        eT[:, 512 * hp:512 * (hp + 1)],
                        start=(jc == 0), stop=(jc == 3),
                        tile_position=(0, 32 * hp), skip_group_check=True)

            def emit_lepe(cur):
                # 9-tap depthwise conv on DVE over the padded vT layout
                vp = cur.t["vp"]
                acc = sp.tile([P, VPD], BF16, name=f"ac{cur.w}{cur.g}",
                              tag="acc", bufs=2)
                for tap in range(9):
                    dy, dx = tap // 3 - 1, tap % 3 - 1
                    so = GUARD + SEG * dx + dy
                    src = vp[:, cur.g, so:so + VPD]
                    wcol = ld_sb[:, 9 * cur.g + tap:9 * cur.g + tap + 1]
                    if tap == 0:
                        nc.vector.tensor_scalar_mul(acc[:], src, wcol)
                    else:
                        nc.vector.scalar_tensor_tensor(
                            out=acc[:], in0=src, scalar=wcol, in1=acc[:],
                            op0=MUL, op1=ADD)
                cur.acc = acc

            def emit_tail(prev):
                # DVE: denominators + merge (pv/den + lepe) -> mg (bf16)
                rbs = sp.tile([P, WIN], F32, name=f"rb{prev.w}{prev.g}",
                              tag="rbs", bufs=2)
                nc.vector.reciprocal_approx_fast(rbs[:], prev.sm[:])
                tmp = sp.tile([P, WIN], F32, name=f"tm{prev.w}{prev.g}",
                              tag="tmp", bufs=2)
                nc.vector.tensor_tensor(out=tmp[:], in0=prev.pv[:],
                                        in1=rbs[:], op=MUL)
                mg = sp.tile([P, WIN], BF16, name=f"mg{prev.w}{prev.g}",
                             tag="mg", bufs=4)
                nc.vector.tensor_tensor(
                    out=mg[:].rearrange("p (s x) -> p s x", s=STRIPE),
                    in0=tmp[:].rearrange("p (s x) -> p s x", s=STRIPE),
                    in1=prev.acc[:].rearrange(
                        "p (s x) -> p s x", s=STRIPE)[:, :, :RESO],
                    op=ADD)
                prev.mg = mg

            def emit_proj(w, mg0, mg1):
                for t4 in range(4):
                    pj = pp.tile([P, DIM], F32, name=f"pj{w}{t4}",
                                 tag="pj", bufs=2)
                    nc.tensor.matmul(pj[:], mg0[:, P * t4:P * (t4 + 1)],
                                     pw_sb[:, 0, :], start=True, stop=False)
                    nc.tensor.matmul(pj[:], mg1[:, P * t4:P * (t4 + 1)],
                                     pw_sb[:, 1, :], start=False, stop=False)
                    nc.tensor.matmul(pj[:], ones_row[:], pb_sb[:],
                                     start=False, stop=True)
                    ob = sp.tile([P, DIM], F32, name=f"ob{w}{t4}",
                                 tag="ob", bufs=3)
                    nc.vector.tensor_copy(ob[:], pj[:])
                    for s1 in range(2):
                        nc.sync.dma_start(ov[w, t4, s1],
                                          ob[RESO * s1:RESO * (s1 + 1), :])

            # ---- software-pipelined main loop ----
            mgs = {}
            tiles = load_w(0)
            prev = None
            for w in range(NW):
                for g in (0, 1):
                    cur = Slot(w, g, tiles)
                    emit_qk(cur, 0)
                    if prev is not None:
                        emit_pvsm(prev, 1)
                    emit_qk(cur, 1)
                    if prev is not None:
                        emit_pvsm(prev, 2)
                    emit_qk(cur, 2)
                    if prev is not None:
                        emit_pvsm(prev, 3)
                        emit_tail(prev)
                        mgs.setdefault(prev.w, {})[prev.g] = prev.mg
                    emit_qk(cur, 3)
                    if prev is not None and prev.g == 1:
                        m = mgs.pop(prev.w)
                        emit_proj(prev.w, m[0], m[1])
                    emit_pvsm(cur, 0)
                    emit_lepe(cur)
                    prev = cur
                if w + 1 < NW:
                    tiles = load_w(w + 1)
            # drain
            for jc in (1, 2, 3):
                emit_pvsm(prev, jc)
            emit_tail(prev)
            mgs.setdefault(prev.w, {})[prev.g] = prev.mg
            m = mgs.pop(prev.w)
            emit_proj(prev.w, m[0], m[1])
    return nc


_CACHE = {}


def _get_nc():
    if "nc" not in _CACHE:
        nc = build_nc()
        nc.finalize()
        _CACHE["nc"] = nc
    return _CACHE["nc"]


def _host_prep(qkv, scale, proj_w, proj_b, conv_w, conv_b):
    """Per-core input maps (host-side transposes + weight folding)."""
    scale_v = float(np.asarray(scale).reshape(-1)[0])
    q_all = np.asarray(qkv[0], dtype=np.float32) * scale_v
    k_all = np.asarray(qkv[1], dtype=np.float32)
    v_all = np.asarray(qkv[2], dtype=np.float32)

    bf = ml_dtypes.bfloat16
    # [b, l, c] -> [b, h, w, s2, s1, c]
    def win(x):
        return x.reshape(B, RESO, NW, 4, 2, DIM)

    qw, kw, vw = win(q_all), win(k_all), win(v_all)
    # qT/kT: [b, w, g*128c, j = s2*128 + s1*64 + h]
    qT_h = np.ascontiguousarray(qw.transpose(0, 2, 5, 3, 4, 1)).reshape(
        B, NW, 2, P, WIN).astype(bf)
    kT_h = np.ascontiguousarray(kw.transpose(0, 2, 5, 3, 4, 1)).reshape(
        B, NW, 2, P, WIN).astype(bf)
    # vn: [b, w, p = s1*64 + h, jc = s2, c]
    vn_h = np.ascontiguousarray(vw.transpose(0, 2, 4, 1, 3, 5)).reshape(
        B, NW, P, 4, DIM).astype(bf)
    # vT padded: [b, w, g, c, 66 + s*65 + h], s = s2*2 + s1
    vT_h = np.zeros((B, NW, 2, P, VPT), np.float32)
    vtmp = vw.transpose(0, 2, 5, 3, 4, 1)  # [b, w, c, s2, s1, h]
    vT_h[..., GUARD:GUARD + VPD].reshape(
        B, NW, 2, P, STRIPE, SEG)[..., :RESO] = vtmp.reshape(
        B, NW, 2, P, STRIPE, RESO)
    vT_h = vT_h.astype(bf)

    pw_h = np.ascontiguousarray(np.asarray(proj_w).T).astype(bf)
    # fold conv bias through the projection: out += (conv_b @ proj_w.T)
    pb_h = (np.asarray(proj_b) +
            np.asarray(conv_b) @ np.asarray(proj_w).T).astype(bf)
    pb_h = pb_h.reshape(1, DIM)
    # LePE tap weights: ld[c, 9*g + tap] = conv_w[128*g + c, tap//3, tap%3]
    cw = np.asarray(conv_w).reshape(DIM, 9)
    ld_h = np.zeros((P, 18), np.float32)
    for g in range(2):
        ld_h[:, 9 * g:9 * g + 9] = cw[P * g:P * (g + 1), :]

    in_maps = []
    for b in range(B):
        in_maps.append({
            "qT": np.ascontiguousarray(qT_h[b]),
            "kT": np.ascontiguousarray(kT_h[b]),
            "vn": np.ascontiguousarray(vn_h[b]),
            "vT": np.ascontiguousarray(vT_h[b]),
            "pw": pw_h, "pb": pb_h, "ld": ld_h,
        })
    return in_maps


LAST_RESULTS = None


def kernel(qkv, scale, proj_w, proj_b, conv_w, conv_b):
    global LAST_RESULTS
    from concourse.bass_utils import run_bass_kernel_spmd
    nc = _get_nc()
    in_maps = _host_prep(qkv, scale, proj_w, proj_b, conv_w, conv_b)
    res = run_bass_kernel_spmd(nc, in_maps, core_ids=list(range(B)))
    LAST_RESULTS = res
    outs = [np.asarray(res.results[b]["out"], dtype=np.float32) for b in range(B)]
    return np.stack(outs, axis=0)
